# revision 29
# baseline (speedup 1.0000x reference)
"""Trainium2 Bass kernel for nn_ConstraintsModule (v2).

Reference math:
    m = preds[:, atoms]                                   # [B, N]
    body_rev[b,c,j] = pos_body[c,j] + m[b,j]*(neg_body-pos_body)[c,j]
    body_min[b,c]   = 1 - max_j body_rev[b,c,j]
    lb[b,n] = max_c body_min[b,c]*pos_head[c,n]
    ub[b,n] = 1 - max_c body_min[b,c]*neg_head[c,n]
    updated = clamp(m, min(lb,ub), max(lb,ub))
    out = preds with columns `atoms` replaced by updated

Key rewrite: body_min[b,c] = min( min_{j in pos(c)} m[b,j],
                                  min_{j in neg(c)} 1-m[b,j] ).
The host packs, per constraint slot, the fp16 literal values (m for pos
literals, 1-m for neg, 1.0 pad) with slots sorted into width tiers, so
the device does ONE min-reduce per tier to get body_min directly.
Packing m (not 1-m) for pos literals keeps body_min's RELATIVE error at
~2^-11, so tiny expected outputs stay accurate.

Head phase: body_min columns are permuted into head-bin order with a
one-hot matmul on the idle PE (transpose body_min, multiply by a 0/1
selection matrix P), then two small grouped max-reduces produce
lb / ubm per head atom.  The final clamp against exact fp32 m runs on
the host (elementwise glue, like the gather/scatter).

Sharding: constraints live on the core that owns their head atom; atoms
are dealt greedily to balance per-tier slot counts. All 128 batch rows
sit on the SBUF partition axis; all cores share one SPMD program.
"""

import sys
from contextlib import ExitStack

import numpy as np

if "/opt/trn_rl_repo" not in sys.path:
    sys.path.insert(0, "/opt/trn_rl_repo")

import concourse.bacc as bacc
import concourse.tile as tile
from concourse import mybir
from concourse.bass_utils import run_bass_kernel_spmd
from concourse.masks import make_identity

B = 128
C = 1024
N = 512
NCORES = 8
# graded DMA chunk fractions: small first so the first min-reduce can
# start as early as possible, but few and large — queue throughput scales
# with per-partition line size (~700B lines run at half the rate of ~2KB
# lines).  Even indices ride scalar's queue, odd ride gpsimd's.
CHUNK_FRACS = (0.11, 0.20, 0.24, 0.25, 0.20)

# Set by test.py to profile; the grading path leaves these alone.
_TRACE = False
_LAST_RESULTS = None

_PROGRAM_CACHE: dict = {}


# --------------------------------------------------------------------------
# host-side planning
# --------------------------------------------------------------------------

def _build_plan(pos_head, neg_head, pos_body, neg_body):
    pb = pos_body != 0
    nb = neg_body != 0
    W = (pb.sum(1) + nb.sum(1)).astype(np.int64)

    ph_atom = pos_head.argmax(1)
    ph_has = pos_head.max(1) > 0
    nh_atom = neg_head.argmax(1)
    nh_has = neg_head.max(1) > 0
    pos_bins = [[] for _ in range(N)]
    neg_bins = [[] for _ in range(N)]
    for c in np.nonzero(ph_has)[0]:
        pos_bins[ph_atom[c]].append(int(c))
    for c in np.nonzero(nh_has)[0]:
        neg_bins[nh_atom[c]].append(int(c))
    head_atoms = [n for n in range(N) if pos_bins[n] or neg_bins[n]]

    # tier widths via DP over the W histogram (even candidate widths)
    def r2(x):
        return (x + 1) // 2 * 2

    cands = sorted({r2(int(w)) for w in W})
    PEN = 260
    nc_ = len(cands)
    counts = np.zeros(nc_, np.int64)
    for w in W:
        counts[np.searchsorted(cands, r2(int(w)))] += 1
    csum = np.concatenate([[0], np.cumsum(counts)])
    wsum = np.concatenate([[0], np.cumsum(counts * np.array(cands))])
    f = np.full(nc_, 1 << 60)
    arg = [None] * nc_
    for i in range(nc_):
        for j in range(-1, i):
            cost = cands[i] * (csum[i + 1] - csum[j + 1]) - (
                wsum[i + 1] - wsum[j + 1]
            )
            base = 0 if j < 0 else f[j]
            if base + cost + PEN < f[i]:
                f[i] = base + cost + PEN
                arg[i] = j
    tiers = []
    i = nc_ - 1
    while i >= 0:
        tiers.append(cands[i])
        i = arg[i]
        if i is None:
            break
    tier_ws = sorted(tiers)
    NT = len(tier_ws)
    tier_of = np.searchsorted(tier_ws, [r2(int(w)) for w in W])

    # head group buckets: small uniform group + catch-all group
    sp = np.array([len(pos_bins[n]) for n in range(N)])
    sn = np.array([len(neg_bins[n]) for n in range(N)])
    SPmax, SNmax = int(sp.max()), int(sn.max())
    best = None
    for s1 in (1, 2, 3, 4):
        for n1 in (1, 2):
            g1 = [n for n in head_atoms if sp[n] <= s1 and sn[n] <= n1]
            g2 = [n for n in head_atoms if not (sp[n] <= s1 and sn[n] <= n1)]
            c1 = -(-len(g1) // NCORES) if g1 else 0
            c2 = -(-len(g2) // NCORES) if g2 else 0
            T = c1 * (s1 + n1) + c2 * (SPmax + SNmax)
            cost = T + 450 * (2 if g2 else 1)
            if T <= 512 and (best is None or cost < best[0]):
                best = (cost, s1, n1)
    _, SP1, SN1 = best
    grp_of = {
        n: 0 if (sp[n] <= SP1 and sn[n] <= SN1) else 1 for n in head_atoms
    }

    # atom -> core greedy assignment balancing count/tiers/groups
    sz = {n: int(sp[n] + sn[n]) for n in head_atoms}
    tvec = {}
    for n in head_atoms:
        v = np.zeros(NT, np.int64)
        for cid in pos_bins[n] + neg_bins[n]:
            v[tier_of[cid]] += 1
        tvec[n] = v
    order = sorted(head_atoms, key=lambda n: (-sz[n], n))
    cnt = np.zeros(NCORES, np.int64)
    tc = np.zeros((NCORES, NT), np.int64)
    gc = np.zeros((NCORES, 2), np.int64)
    core_of = {}
    tws = np.array(tier_ws, np.float64)
    for n in order:
        best_s, best_core = None, 0
        curmax = tc.max(0)
        for k in range(NCORES):
            over = max(0, cnt[k] + sz[n] - C // NCORES) * 1e9
            newmax = np.maximum(tc[k] + tvec[n], curmax)
            s = (
                over
                + float((tws * (newmax - curmax)).sum())
                + 5.0 * gc[k][grp_of[n]]
                + 0.1 * cnt[k]
            )
            if best_s is None or s < best_s:
                best_s, best_core = s, k
        core_of[n] = best_core
        cnt[best_core] += sz[n]
        tc[best_core] += tvec[n]
        gc[best_core][grp_of[n]] += 1

    # per-core constraint tiering with promotion smoothing
    targ = [int(-(-int(tc[:, t].sum()) // NCORES)) for t in range(NT)]
    core_tier_cons = [[[] for _ in range(NT)] for _ in range(NCORES)]
    for n in head_atoms:
        k = core_of[n]
        for cid in pos_bins[n] + neg_bins[n]:
            core_tier_cons[k][tier_of[cid]].append(cid)
    for k in range(NCORES):
        for t in range(NT - 1):
            ex = len(core_tier_cons[k][t]) - targ[t]
            if ex > 0:
                moved = core_tier_cons[k][t][-ex:]
                core_tier_cons[k][t] = core_tier_cons[k][t][:-ex]
                core_tier_cons[k][t + 1] = moved + core_tier_cons[k][t + 1]
    Ct = [max(len(core_tier_cons[k][t]) for k in range(NCORES)) for t in range(NT)]
    # squeeze slot count when a tier-cap decrement is free (max-count cores
    # can promote into slack of the next tier); the transpose/matmul path
    # chunks over slot ranges, so SPAD slightly above 128 is fine.
    changed = True
    while changed:
        changed = False
        for t in range(NT - 1):
            if Ct[t] == 0:
                continue
            ok = all(
                len(core_tier_cons[k][t]) < Ct[t]
                or len(core_tier_cons[k][t + 1]) + 1 <= Ct[t + 1]
                for k in range(NCORES)
            )
            if ok:
                for k in range(NCORES):
                    if len(core_tier_cons[k][t]) == Ct[t]:
                        cid = core_tier_cons[k][t].pop()
                        core_tier_cons[k][t + 1].insert(0, cid)
                Ct[t] -= 1
                changed = True
    SPAD = sum(Ct)
    KCH = -(-SPAD // 128)

    C1 = max(int(gc[k][0]) for k in range(NCORES))
    C2 = max(int(gc[k][1]) for k in range(NCORES))
    W1, W2 = SP1 + SN1, SPmax + SNmax
    T1 = C1 * W1
    T = T1 + C2 * W2
    NAcap = C1 + C2

    GL = sum(Ct[t] * tier_ws[t] for t in range(NT))

    # DMA chunk plan: graded (small first) chunks; boundaries land on slot
    # boundaries, possibly mid-tier.  Each chunk is a list of "runs"
    # (tier, slot0, slot1) — one min-reduce per run, gated by its chunk.
    bounds = []
    acc = 0.0
    for fr in CHUNK_FRACS[:-1]:
        acc += fr
        bounds.append(acc * GL)
    chunks = []  # list of (col0, col1, runs) with runs=(wt, col, s0, s1, sidx)
    cur_runs = []
    cur_col0 = 0
    col = 0
    done = 0
    si = 0
    bi = 0
    for t in range(NT):
        wt = tier_ws[t]
        s = 0
        while s < Ct[t]:
            budget = (bounds[bi] - done) if bi < len(bounds) else (GL - done)
            take = min(max(int(budget // wt), 1), Ct[t] - s)
            cur_runs.append((wt, col + s * wt, s, s + take, si + s))
            s += take
            done += take * wt
            if bi < len(bounds) and done >= bounds[bi] - wt:
                chunks.append((cur_col0, col + s * wt, tuple(cur_runs)))
                cur_col0 = col + s * wt
                cur_runs = []
                bi += 1
        col += Ct[t] * wt
        si += Ct[t]
    if cur_runs:
        chunks.append((cur_col0, GL, tuple(cur_runs)))

    # transpose/matmul k-chunks: (s0, s1) slot ranges, each <=128 wide,
    # split near the middle at a run boundary so the first
    # transpose/copy/matmul overlaps the remaining reduces.
    assert SPAD <= 256
    half = SPAD // 2
    best_run_end = 0
    for _, _, runs in chunks:
        for (wt, rcol, s0, s1, sidx) in runs:
            e = sidx + (s1 - s0)
            if 0 < e < SPAD and abs(e - half) < abs(best_run_end - half):
                best_run_end = e
    if 0 < best_run_end < SPAD and SPAD - best_run_end <= 128 and best_run_end <= 128:
        ksplits = ((0, best_run_end), (best_run_end, SPAD))
    elif SPAD <= 128:
        ksplits = ((0, SPAD),)
    else:
        ksplits = ((0, 128), (128, SPAD))

    PAD1 = 2 * N
    cores = []
    for k in range(NCORES):
        slot_of = {}
        gidx = np.full((GL,), PAD1, np.int32)
        off = si = 0
        for t in range(NT):
            wt = tier_ws[t]
            for j, cid in enumerate(core_tier_cons[k][t]):
                slot_of[cid] = si + j
                jp = np.nonzero(pb[cid])[0]
                jn = np.nonzero(nb[cid])[0]
                row = off + j * wt
                gidx[row : row + jp.size] = jp            # value m
                gidx[row + jp.size : row + jp.size + jn.size] = N + jn  # 1-m
            off += Ct[t] * wt
            si += Ct[t]
        atoms_g = [[], []]
        for n in head_atoms:
            if core_of[n] == k:
                atoms_g[grp_of[n]].append(n)
        NK = len(ksplits)
        kblk_of = np.zeros(SPAD, np.int64)
        for i, (ks0, ks1) in enumerate(ksplits):
            kblk_of[ks0:ks1] = i
        P = np.zeros((128, NK * T), np.float16)

        def pset(s, colt):
            i = kblk_of[s]
            P[s - ksplits[i][0], i * T + colt] = 1.0

        lb_cols = []
        ub_cols = []
        out_aids = []
        for g, (cap, wg, spg) in enumerate([(C1, W1, SP1), (C2, W2, SPmax)]):
            base0 = 0 if g == 0 else T1
            for i, n in enumerate(atoms_g[g]):
                base = base0 + i * wg
                for l, cid in enumerate(pos_bins[n]):
                    pset(slot_of[cid], base + l)
                for l, cid in enumerate(neg_bins[n]):
                    pset(slot_of[cid], base + spg + l)
                acol0 = 0 if g == 0 else 2 * C1
                sng = SN1 if g == 0 else SNmax
                cg = C1 if g == 0 else C2
                if spg == sng:  # device emits one interleaved reduce
                    lb_cols.append(acol0 + 2 * i)
                    ub_cols.append(acol0 + 2 * i + 1)
                else:
                    lb_cols.append(acol0 + i)
                    ub_cols.append(acol0 + cg + i)
                out_aids.append(n)
        cores.append(
            dict(
                gidx=gidx,
                P=P,
                lb_cols=np.array(lb_cols, np.int64),
                ub_cols=np.array(ub_cols, np.int64),
                out_aids=np.array(out_aids, np.int64),
            )
        )

    dims = (
        tuple(tier_ws),
        tuple(Ct),
        SPAD,
        KCH,
        (SP1, SN1, C1),
        (SPmax, SNmax, C2),
        GL,
        T,
        T1,
        NAcap,
        tuple(chunks),
        ksplits,
    )
    return dims, cores


# --------------------------------------------------------------------------
# device program
# --------------------------------------------------------------------------

def _build_program(dims):
    if dims in _PROGRAM_CACHE:
        return _PROGRAM_CACHE[dims]
    (tier_ws, Ct, SPAD, KCH, g1, g2, GL, T, T1, NAc, chunks, ksplits) = dims
    dt = mybir.dt
    NK = len(ksplits)

    nc = bacc.Bacc(
        "TRN2", target_bir_lowering=False, debug=False, enable_partition_id=False
    )
    g_d = nc.dram_tensor("g", [B, GL], dt.float16, kind="ExternalInput")
    p_d = nc.dram_tensor("p", [128, NK * T], dt.float16, kind="ExternalInput")
    o_d = nc.dram_tensor("lbub", [B, 2 * NAc], dt.float16, kind="ExternalOutput")

    with ExitStack() as ctx:
        tc = ctx.enter_context(tile.TileContext(nc))
        pool = ctx.enter_context(tc.tile_pool(name="main", bufs=1))
        psum = ctx.enter_context(tc.tile_pool(name="ps", bufs=1, space="PSUM"))

        g_sb = pool.tile([B, GL], dt.float16, tag="g")
        p_sb = pool.tile([128, NK * T], dt.float16, tag="p")
        ident = pool.tile([128, 128], dt.float16, tag="ident")
        bmin = pool.tile([B, SPAD], dt.float16, tag="bmin")
        bminT = pool.tile([128, NK * 128], dt.float16, tag="bminT")
        lbub = pool.tile([B, 2 * NAc], dt.float16, tag="lbub")

        # Only two HWDGE queues ever run (a third steals bandwidth share):
        # G chunks alternate scalar/gpsimd.  P goes as ONE full contiguous
        # tile (sliced/partition-strided P transfers generate small DMA
        # packets that wreck the queue), late in scalar's stream but ahead
        # of its last G chunk so it lands before the permute matmuls.
        dma_engines = [nc.scalar, nc.gpsimd]
        last_gp = max(i for i in range(len(chunks)) if i % 2 == 1)
        for i, (col0, col1, runs) in enumerate(chunks):
            if i == last_gp:
                nc.gpsimd.dma_start(p_sb[:], p_d.ap())
            eng = dma_engines[i % 2]
            eng.dma_start(g_sb[:, col0:col1], g_d.ap()[:, col0:col1])
        make_identity(nc, ident[:])

        # body phase: one min-reduce per run, gated by its chunk's DMA
        for (col0, col1, runs) in chunks:
            for (wt, rcol, s0, s1, sidx) in runs:
                g3 = g_sb[:, rcol : rcol + (s1 - s0) * wt].rearrange(
                    "p (c k) -> p c k", k=wt
                )
                nc.vector.tensor_reduce(
                    bmin[:, sidx : sidx + (s1 - s0)],
                    g3,
                    axis=mybir.AxisListType.X,
                    op=mybir.AluOpType.min,
                )

        # permute into head-bin order: transpose + one-hot matmul on PE,
        # chunked over slot ranges so early chunks overlap late reduces
        hp = psum.tile([B, T], dt.float32, tag="hp")
        for i, (ks0, ks1) in enumerate(ksplits):
            k = ks1 - ks0
            psT = psum.tile([128, 128], dt.float16, tag=f"psT{i}")
            nc.tensor.transpose(psT[0:k, :], bmin[:, ks0:ks1], ident[:])
            # last chunk's copy rides the (now idle) vector engine: it sits
            # on the critical path after the final body reduce
            if i == NK - 1:
                nc.vector.tensor_copy(
                    bminT[0:k, i * 128 : i * 128 + 128], psT[0:k, :]
                )
            else:
                nc.scalar.copy(bminT[0:k, i * 128 : i * 128 + 128], psT[0:k, :])
            nc.tensor.matmul(
                hp[:],
                bminT[0:k, i * 128 : i * 128 + 128],
                p_sb[0:k, i * T : (i + 1) * T],
                start=(i == 0),
                stop=(i == NK - 1),
            )

        # head phase: grouped segment maxes -> lb | ubm.  When a group's
        # pos/neg widths match, lb and ubm come out of ONE reduce with
        # interleaved columns (handled by the host unpack).
        (SP1, SN1, C1) = g1
        (SP2, SN2, C2) = g2
        acol = 0
        out_engines = [nc.scalar, nc.sync]
        for gi, (spg, sng, cg, base) in enumerate(
            ((SP1, SN1, C1, 0), (SP2, SN2, C2, T1))
        ):
            if cg == 0:
                continue
            wg = spg + sng
            if spg == sng:
                seg = hp[:, base : base + cg * wg].rearrange(
                    "p (a l) -> p a l", l=spg
                )
                nc.vector.tensor_reduce(
                    lbub[:, acol : acol + 2 * cg],
                    seg,
                    axis=mybir.AxisListType.X,
                    op=mybir.AluOpType.max,
                )
            else:
                seg = hp[:, base : base + cg * wg].rearrange(
                    "p (a l) -> p a l", l=wg
                )
                nc.vector.tensor_reduce(
                    lbub[:, acol : acol + cg],
                    seg[:, :, 0:spg],
                    axis=mybir.AxisListType.X,
                    op=mybir.AluOpType.max,
                )
                nc.vector.tensor_reduce(
                    lbub[:, acol + cg : acol + 2 * cg],
                    seg[:, :, spg:wg],
                    axis=mybir.AxisListType.X,
                    op=mybir.AluOpType.max,
                )
            acol += 2 * cg

        nc.scalar.dma_start(o_d.ap()[:, 0:acol], lbub[:, 0:acol])

    nc.compile()
    _PROGRAM_CACHE[dims] = nc
    return nc


# --------------------------------------------------------------------------
# entry point
# --------------------------------------------------------------------------

def kernel(preds, pos_head, neg_head, pos_body, neg_body, atoms):
    global _LAST_RESULTS
    preds = np.ascontiguousarray(np.asarray(preds, dtype=np.float32))
    pos_head = np.asarray(pos_head)
    neg_head = np.asarray(neg_head)
    pos_body = np.asarray(pos_body)
    neg_body = np.asarray(neg_body)
    atoms_np = np.asarray(atoms).astype(np.int64)

    dims, cores = _build_plan(pos_head, neg_head, pos_body, neg_body)
    nc = _build_program(dims)

    m = np.ascontiguousarray(preds[:, atoms_np])  # [B, N] fp32
    m16 = m.astype(np.float16)
    om16 = (np.float32(1.0) - m).astype(np.float16)
    m2 = np.concatenate([m16, om16, np.ones((B, 1), np.float16)], axis=1)

    in_maps = []
    for k in range(NCORES):
        in_maps.append(
            {
                "g": np.ascontiguousarray(m2[:, cores[k]["gidx"]]),
                "p": np.ascontiguousarray(cores[k]["P"]),
            }
        )

    res = run_bass_kernel_spmd(
        nc, in_maps, core_ids=list(range(NCORES)), trace=_TRACE
    )
    _LAST_RESULTS = res

    out = preds.copy()
    for k in range(NCORES):
        r = np.asarray(res.results[k]["lbub"])
        aids = cores[k]["out_aids"]
        if not len(aids):
            continue
        lb = r[:, cores[k]["lb_cols"]].astype(np.float32)
        ubm = r[:, cores[k]["ub_cols"]].astype(np.float32)
        ub = np.float32(1.0) - ubm
        lo = np.minimum(lb, ub)
        hi = np.maximum(lb, ub)
        mm = m[:, aids]
        upd = np.maximum(lo, np.minimum(hi, mm))
        out[:, atoms_np[aids]] = upd
    return out


# revision 30
# speedup vs baseline: 1.0354x; 1.0354x over previous
"""Trainium2 Bass kernel for nn_ConstraintsModule (v2).

Reference math:
    m = preds[:, atoms]                                   # [B, N]
    body_rev[b,c,j] = pos_body[c,j] + m[b,j]*(neg_body-pos_body)[c,j]
    body_min[b,c]   = 1 - max_j body_rev[b,c,j]
    lb[b,n] = max_c body_min[b,c]*pos_head[c,n]
    ub[b,n] = 1 - max_c body_min[b,c]*neg_head[c,n]
    updated = clamp(m, min(lb,ub), max(lb,ub))
    out = preds with columns `atoms` replaced by updated

Key rewrite: body_min[b,c] = min( min_{j in pos(c)} m[b,j],
                                  min_{j in neg(c)} 1-m[b,j] ).
The host packs, per constraint slot, the fp16 literal values (m for pos
literals, 1-m for neg, 1.0 pad) with slots sorted into width tiers, so
the device does ONE min-reduce per tier to get body_min directly.
Packing m (not 1-m) for pos literals keeps body_min's RELATIVE error at
~2^-11, so tiny expected outputs stay accurate.

Head phase: body_min columns are permuted into head-bin order with a
one-hot matmul on the idle PE (transpose body_min, multiply by a 0/1
selection matrix P), then two small grouped max-reduces produce
lb / ubm per head atom.  The final clamp against exact fp32 m runs on
the host (elementwise glue, like the gather/scatter).

Sharding: constraints live on the core that owns their head atom; atoms
are dealt greedily to balance per-tier slot counts. All 128 batch rows
sit on the SBUF partition axis; all cores share one SPMD program.
"""

import sys
from contextlib import ExitStack

import numpy as np

if "/opt/trn_rl_repo" not in sys.path:
    sys.path.insert(0, "/opt/trn_rl_repo")

import concourse.bacc as bacc
import concourse.tile as tile
from concourse import mybir
from concourse.bass_utils import run_bass_kernel_spmd
from concourse.masks import make_identity

B = 128
C = 1024
N = 512
NCORES = 8
# graded DMA chunk fractions: small first so the first min-reduce can
# start as early as possible, but few and large — queue throughput scales
# with per-partition line size (~700B lines run at half the rate of ~2KB
# lines).  Even indices ride scalar's queue, odd ride gpsimd's.
CHUNK_FRACS = (0.14, 0.20, 0.22, 0.22, 0.22)

# Set by test.py to profile; the grading path leaves these alone.
_TRACE = False
_LAST_RESULTS = None

_PROGRAM_CACHE: dict = {}


# --------------------------------------------------------------------------
# host-side planning
# --------------------------------------------------------------------------

def _build_plan(pos_head, neg_head, pos_body, neg_body):
    pb = pos_body != 0
    nb = neg_body != 0
    W = (pb.sum(1) + nb.sum(1)).astype(np.int64)

    ph_atom = pos_head.argmax(1)
    ph_has = pos_head.max(1) > 0
    nh_atom = neg_head.argmax(1)
    nh_has = neg_head.max(1) > 0
    pos_bins = [[] for _ in range(N)]
    neg_bins = [[] for _ in range(N)]
    for c in np.nonzero(ph_has)[0]:
        pos_bins[ph_atom[c]].append(int(c))
    for c in np.nonzero(nh_has)[0]:
        neg_bins[nh_atom[c]].append(int(c))
    head_atoms = [n for n in range(N) if pos_bins[n] or neg_bins[n]]

    # tier widths via DP over the W histogram (even candidate widths)
    def r2(x):
        return (x + 1) // 2 * 2

    cands = sorted({r2(int(w)) for w in W})
    PEN = 260
    nc_ = len(cands)
    counts = np.zeros(nc_, np.int64)
    for w in W:
        counts[np.searchsorted(cands, r2(int(w)))] += 1
    csum = np.concatenate([[0], np.cumsum(counts)])
    wsum = np.concatenate([[0], np.cumsum(counts * np.array(cands))])
    f = np.full(nc_, 1 << 60)
    arg = [None] * nc_
    for i in range(nc_):
        for j in range(-1, i):
            cost = cands[i] * (csum[i + 1] - csum[j + 1]) - (
                wsum[i + 1] - wsum[j + 1]
            )
            base = 0 if j < 0 else f[j]
            if base + cost + PEN < f[i]:
                f[i] = base + cost + PEN
                arg[i] = j
    tiers = []
    i = nc_ - 1
    while i >= 0:
        tiers.append(cands[i])
        i = arg[i]
        if i is None:
            break
    tier_ws = sorted(tiers)
    NT = len(tier_ws)
    tier_of = np.searchsorted(tier_ws, [r2(int(w)) for w in W])

    # head group buckets: small uniform group + catch-all group
    sp = np.array([len(pos_bins[n]) for n in range(N)])
    sn = np.array([len(neg_bins[n]) for n in range(N)])
    SPmax, SNmax = int(sp.max()), int(sn.max())
    best = None
    for s1 in (1, 2, 3, 4):
        for n1 in (1, 2):
            g1 = [n for n in head_atoms if sp[n] <= s1 and sn[n] <= n1]
            g2 = [n for n in head_atoms if not (sp[n] <= s1 and sn[n] <= n1)]
            c1 = -(-len(g1) // NCORES) if g1 else 0
            c2 = -(-len(g2) // NCORES) if g2 else 0
            T = c1 * (s1 + n1) + c2 * (SPmax + SNmax)
            cost = T + 450 * (2 if g2 else 1)
            if T <= 512 and (best is None or cost < best[0]):
                best = (cost, s1, n1)
    _, SP1, SN1 = best
    grp_of = {
        n: 0 if (sp[n] <= SP1 and sn[n] <= SN1) else 1 for n in head_atoms
    }

    # atom -> core greedy assignment balancing count/tiers/groups
    sz = {n: int(sp[n] + sn[n]) for n in head_atoms}
    tvec = {}
    for n in head_atoms:
        v = np.zeros(NT, np.int64)
        for cid in pos_bins[n] + neg_bins[n]:
            v[tier_of[cid]] += 1
        tvec[n] = v
    order = sorted(head_atoms, key=lambda n: (-sz[n], n))
    cnt = np.zeros(NCORES, np.int64)
    tc = np.zeros((NCORES, NT), np.int64)
    gc = np.zeros((NCORES, 2), np.int64)
    core_of = {}
    tws = np.array(tier_ws, np.float64)
    for n in order:
        best_s, best_core = None, 0
        curmax = tc.max(0)
        for k in range(NCORES):
            over = max(0, cnt[k] + sz[n] - C // NCORES) * 1e9
            newmax = np.maximum(tc[k] + tvec[n], curmax)
            s = (
                over
                + float((tws * (newmax - curmax)).sum())
                + 5.0 * gc[k][grp_of[n]]
                + 0.1 * cnt[k]
            )
            if best_s is None or s < best_s:
                best_s, best_core = s, k
        core_of[n] = best_core
        cnt[best_core] += sz[n]
        tc[best_core] += tvec[n]
        gc[best_core][grp_of[n]] += 1

    # per-core constraint tiering with promotion smoothing
    targ = [int(-(-int(tc[:, t].sum()) // NCORES)) for t in range(NT)]
    core_tier_cons = [[[] for _ in range(NT)] for _ in range(NCORES)]
    for n in head_atoms:
        k = core_of[n]
        for cid in pos_bins[n] + neg_bins[n]:
            core_tier_cons[k][tier_of[cid]].append(cid)
    for k in range(NCORES):
        for t in range(NT - 1):
            ex = len(core_tier_cons[k][t]) - targ[t]
            if ex > 0:
                moved = core_tier_cons[k][t][-ex:]
                core_tier_cons[k][t] = core_tier_cons[k][t][:-ex]
                core_tier_cons[k][t + 1] = moved + core_tier_cons[k][t + 1]
    Ct = [max(len(core_tier_cons[k][t]) for k in range(NCORES)) for t in range(NT)]
    # squeeze slot count when a tier-cap decrement is free (max-count cores
    # can promote into slack of the next tier); the transpose/matmul path
    # chunks over slot ranges, so SPAD slightly above 128 is fine.
    changed = True
    while changed:
        changed = False
        for t in range(NT - 1):
            if Ct[t] == 0:
                continue
            ok = all(
                len(core_tier_cons[k][t]) < Ct[t]
                or len(core_tier_cons[k][t + 1]) + 1 <= Ct[t + 1]
                for k in range(NCORES)
            )
            if ok:
                for k in range(NCORES):
                    if len(core_tier_cons[k][t]) == Ct[t]:
                        cid = core_tier_cons[k][t].pop()
                        core_tier_cons[k][t + 1].insert(0, cid)
                Ct[t] -= 1
                changed = True
    SPAD = sum(Ct)
    KCH = -(-SPAD // 128)

    C1 = max(int(gc[k][0]) for k in range(NCORES))
    C2 = max(int(gc[k][1]) for k in range(NCORES))
    W1, W2 = SP1 + SN1, SPmax + SNmax
    T1 = C1 * W1
    T = T1 + C2 * W2
    NAcap = C1 + C2

    GL = sum(Ct[t] * tier_ws[t] for t in range(NT))

    # DMA chunk plan: graded (small first) chunks; boundaries land on slot
    # boundaries, possibly mid-tier.  Each chunk is a list of "runs"
    # (tier, slot0, slot1) — one min-reduce per run, gated by its chunk.
    bounds = []
    acc = 0.0
    for fr in CHUNK_FRACS[:-1]:
        acc += fr
        bounds.append(acc * GL)
    chunks = []  # list of (col0, col1, runs) with runs=(wt, col, s0, s1, sidx)
    cur_runs = []
    cur_col0 = 0
    col = 0
    done = 0
    si = 0
    bi = 0
    for t in range(NT):
        wt = tier_ws[t]
        s = 0
        while s < Ct[t]:
            budget = (bounds[bi] - done) if bi < len(bounds) else (GL - done)
            take = min(max(int(budget // wt), 1), Ct[t] - s)
            cur_runs.append((wt, col + s * wt, s, s + take, si + s))
            s += take
            done += take * wt
            if bi < len(bounds) and done >= bounds[bi] - wt:
                chunks.append((cur_col0, col + s * wt, tuple(cur_runs)))
                cur_col0 = col + s * wt
                cur_runs = []
                bi += 1
        col += Ct[t] * wt
        si += Ct[t]
    if cur_runs:
        chunks.append((cur_col0, GL, tuple(cur_runs)))

    # transpose/matmul k-chunks: (s0, s1) slot ranges, each <=128 wide,
    # split near the middle at a run boundary so the first
    # transpose/copy/matmul overlaps the remaining reduces.
    assert SPAD <= 256
    half = SPAD // 2
    best_run_end = 0
    for _, _, runs in chunks:
        for (wt, rcol, s0, s1, sidx) in runs:
            e = sidx + (s1 - s0)
            if 0 < e < SPAD and abs(e - half) < abs(best_run_end - half):
                best_run_end = e
    if 0 < best_run_end < SPAD and SPAD - best_run_end <= 128 and best_run_end <= 128:
        ksplits = ((0, best_run_end), (best_run_end, SPAD))
    elif SPAD <= 128:
        ksplits = ((0, SPAD),)
    else:
        ksplits = ((0, 128), (128, SPAD))

    PAD1 = 2 * N
    cores = []
    for k in range(NCORES):
        slot_of = {}
        gidx = np.full((GL,), PAD1, np.int32)
        off = si = 0
        for t in range(NT):
            wt = tier_ws[t]
            for j, cid in enumerate(core_tier_cons[k][t]):
                slot_of[cid] = si + j
                jp = np.nonzero(pb[cid])[0]
                jn = np.nonzero(nb[cid])[0]
                row = off + j * wt
                gidx[row : row + jp.size] = jp            # value m
                gidx[row + jp.size : row + jp.size + jn.size] = N + jn  # 1-m
            off += Ct[t] * wt
            si += Ct[t]
        atoms_g = [[], []]
        for n in head_atoms:
            if core_of[n] == k:
                atoms_g[grp_of[n]].append(n)
        NK = len(ksplits)
        KMAX = max(ks1 - ks0 for ks0, ks1 in ksplits)
        kblk_of = np.zeros(SPAD, np.int64)
        for i, (ks0, ks1) in enumerate(ksplits):
            kblk_of[ks0:ks1] = i
        P = np.zeros((KMAX, NK * T), np.float16)

        def pset(s, colt):
            i = kblk_of[s]
            P[s - ksplits[i][0], i * T + colt] = 1.0

        lb_cols = []
        ub_cols = []
        out_aids = []
        for g, (cap, wg, spg) in enumerate([(C1, W1, SP1), (C2, W2, SPmax)]):
            base0 = 0 if g == 0 else T1
            for i, n in enumerate(atoms_g[g]):
                base = base0 + i * wg
                for l, cid in enumerate(pos_bins[n]):
                    pset(slot_of[cid], base + l)
                for l, cid in enumerate(neg_bins[n]):
                    pset(slot_of[cid], base + spg + l)
                acol0 = 0 if g == 0 else 2 * C1
                sng = SN1 if g == 0 else SNmax
                cg = C1 if g == 0 else C2
                if spg == sng:  # device emits one interleaved reduce
                    lb_cols.append(acol0 + 2 * i)
                    ub_cols.append(acol0 + 2 * i + 1)
                else:
                    lb_cols.append(acol0 + i)
                    ub_cols.append(acol0 + cg + i)
                out_aids.append(n)
        cores.append(
            dict(
                gidx=gidx,
                P=P,
                lb_cols=np.array(lb_cols, np.int64),
                ub_cols=np.array(ub_cols, np.int64),
                out_aids=np.array(out_aids, np.int64),
            )
        )

    dims = (
        tuple(tier_ws),
        tuple(Ct),
        SPAD,
        KCH,
        (SP1, SN1, C1),
        (SPmax, SNmax, C2),
        GL,
        T,
        T1,
        NAcap,
        tuple(chunks),
        ksplits,
    )
    return dims, cores


# --------------------------------------------------------------------------
# device program
# --------------------------------------------------------------------------

def _build_program(dims):
    if dims in _PROGRAM_CACHE:
        return _PROGRAM_CACHE[dims]
    (tier_ws, Ct, SPAD, KCH, g1, g2, GL, T, T1, NAc, chunks, ksplits) = dims
    dt = mybir.dt
    NK = len(ksplits)

    KMAX = max(ks1 - ks0 for ks0, ks1 in ksplits)
    nc = bacc.Bacc(
        "TRN2", target_bir_lowering=False, debug=False, enable_partition_id=False
    )
    g_d = nc.dram_tensor("g", [B, GL], dt.float16, kind="ExternalInput")
    p_d = nc.dram_tensor("p", [KMAX, NK * T], dt.float16, kind="ExternalInput")
    o_d = nc.dram_tensor("lbub", [B, 2 * NAc], dt.float16, kind="ExternalOutput")

    with ExitStack() as ctx:
        tc = ctx.enter_context(tile.TileContext(nc))
        pool = ctx.enter_context(tc.tile_pool(name="main", bufs=1))
        psum = ctx.enter_context(tc.tile_pool(name="ps", bufs=1, space="PSUM"))

        g_sb = pool.tile([B, GL], dt.float16, tag="g")
        p_sb = pool.tile([KMAX, NK * T], dt.float16, tag="p")
        ident = pool.tile([128, 128], dt.float16, tag="ident")
        bmin = pool.tile([B, SPAD], dt.float16, tag="bmin")
        bminT = pool.tile([128, NK * 128], dt.float16, tag="bminT")
        lbub = pool.tile([B, 2 * NAc], dt.float16, tag="lbub")

        # Only two HWDGE queues ever run (a third steals bandwidth share):
        # G chunks alternate scalar/gpsimd.  P goes as ONE full contiguous
        # tile (sliced/partition-strided P transfers generate small DMA
        # packets that wreck the queue), late in scalar's stream but ahead
        # of its last G chunk so it lands before the permute matmuls.
        dma_engines = [nc.scalar, nc.gpsimd]
        for i, (col0, col1, runs) in enumerate(chunks):
            eng = dma_engines[i % 2]
            eng.dma_start(g_sb[:, col0:col1], g_d.ap()[:, col0:col1])
        nc.scalar.dma_start(p_sb[:], p_d.ap())
        make_identity(nc, ident[:])

        # body phase: one min-reduce per run, gated by its chunk's DMA
        for (col0, col1, runs) in chunks:
            for (wt, rcol, s0, s1, sidx) in runs:
                g3 = g_sb[:, rcol : rcol + (s1 - s0) * wt].rearrange(
                    "p (c k) -> p c k", k=wt
                )
                nc.vector.tensor_reduce(
                    bmin[:, sidx : sidx + (s1 - s0)],
                    g3,
                    axis=mybir.AxisListType.X,
                    op=mybir.AluOpType.min,
                )

        # permute into head-bin order: transpose + one-hot matmul on PE,
        # chunked over slot ranges so early chunks overlap late reduces
        hp = psum.tile([B, T], dt.float32, tag="hp")
        for i, (ks0, ks1) in enumerate(ksplits):
            k = ks1 - ks0
            psT = psum.tile([128, 128], dt.float16, tag=f"psT{i}")
            nc.tensor.transpose(psT[0:k, :], bmin[:, ks0:ks1], ident[:])
            # last chunk's copy rides the (now idle) vector engine: it sits
            # on the critical path after the final body reduce
            if i == NK - 1:
                nc.vector.tensor_copy(
                    bminT[0:k, i * 128 : i * 128 + 128], psT[0:k, :]
                )
            else:
                nc.scalar.copy(bminT[0:k, i * 128 : i * 128 + 128], psT[0:k, :])
            nc.tensor.matmul(
                hp[:],
                bminT[0:k, i * 128 : i * 128 + 128],
                p_sb[0:k, i * T : (i + 1) * T],
                start=(i == 0),
                stop=(i == NK - 1),
            )

        # head phase: grouped segment maxes -> lb | ubm.  When a group's
        # pos/neg widths match, lb and ubm come out of ONE reduce with
        # interleaved columns (handled by the host unpack).
        (SP1, SN1, C1) = g1
        (SP2, SN2, C2) = g2
        acol = 0
        out_engines = [nc.scalar, nc.sync]
        for gi, (spg, sng, cg, base) in enumerate(
            ((SP1, SN1, C1, 0), (SP2, SN2, C2, T1))
        ):
            if cg == 0:
                continue
            wg = spg + sng
            if spg == sng:
                seg = hp[:, base : base + cg * wg].rearrange(
                    "p (a l) -> p a l", l=spg
                )
                nc.vector.tensor_reduce(
                    lbub[:, acol : acol + 2 * cg],
                    seg,
                    axis=mybir.AxisListType.X,
                    op=mybir.AluOpType.max,
                )
            else:
                seg = hp[:, base : base + cg * wg].rearrange(
                    "p (a l) -> p a l", l=wg
                )
                nc.vector.tensor_reduce(
                    lbub[:, acol : acol + cg],
                    seg[:, :, 0:spg],
                    axis=mybir.AxisListType.X,
                    op=mybir.AluOpType.max,
                )
                nc.vector.tensor_reduce(
                    lbub[:, acol + cg : acol + 2 * cg],
                    seg[:, :, spg:wg],
                    axis=mybir.AxisListType.X,
                    op=mybir.AluOpType.max,
                )
            acol += 2 * cg

        nc.scalar.dma_start(o_d.ap()[:, 0:acol], lbub[:, 0:acol])

    nc.compile()
    _PROGRAM_CACHE[dims] = nc
    return nc


# --------------------------------------------------------------------------
# entry point
# --------------------------------------------------------------------------

def kernel(preds, pos_head, neg_head, pos_body, neg_body, atoms):
    global _LAST_RESULTS
    preds = np.ascontiguousarray(np.asarray(preds, dtype=np.float32))
    pos_head = np.asarray(pos_head)
    neg_head = np.asarray(neg_head)
    pos_body = np.asarray(pos_body)
    neg_body = np.asarray(neg_body)
    atoms_np = np.asarray(atoms).astype(np.int64)

    dims, cores = _build_plan(pos_head, neg_head, pos_body, neg_body)
    nc = _build_program(dims)

    m = np.ascontiguousarray(preds[:, atoms_np])  # [B, N] fp32
    m16 = m.astype(np.float16)
    om16 = (np.float32(1.0) - m).astype(np.float16)
    m2 = np.concatenate([m16, om16, np.ones((B, 1), np.float16)], axis=1)

    in_maps = []
    for k in range(NCORES):
        in_maps.append(
            {
                "g": np.ascontiguousarray(m2[:, cores[k]["gidx"]]),
                "p": np.ascontiguousarray(cores[k]["P"]),
            }
        )

    res = run_bass_kernel_spmd(
        nc, in_maps, core_ids=list(range(NCORES)), trace=_TRACE
    )
    _LAST_RESULTS = res

    out = preds.copy()
    for k in range(NCORES):
        r = np.asarray(res.results[k]["lbub"])
        aids = cores[k]["out_aids"]
        if not len(aids):
            continue
        lb = r[:, cores[k]["lb_cols"]].astype(np.float32)
        ubm = r[:, cores[k]["ub_cols"]].astype(np.float32)
        ub = np.float32(1.0) - ubm
        lo = np.minimum(lb, ub)
        hi = np.maximum(lb, ub)
        mm = m[:, aids]
        upd = np.maximum(lo, np.minimum(hi, mm))
        out[:, atoms_np[aids]] = upd
    return out
